# revision 20
# baseline (speedup 1.0000x reference)
"""Self-contained Trainium2 Bass kernel for nn_AMRModel_48455821033670.

Data-parallel over B*T across 8 NeuronCores (1 dialog per core).
Per core: BERT-layer utterance encoder (1024 tok, CLS-only FFN tail),
BERT-layer node encoder (6144 tok) with fused attention pool, 2 RGAT
layers (dense per-relation matmuls with host-built incidence matrices),
MLP classifier + NLL loss. Token-major bf16 activations, f32 PSUM
accumulation, DMA transposes to feed the PE's lhsT operand.
"""

import contextlib
import os

import numpy as np
import ml_dtypes

import concourse.bass as bass
import concourse.bacc as bacc
import concourse.mybir as mybir
import concourse.tile as tile
from concourse.bass import ds, ts
from concourse.bass_utils import run_bass_kernel_spmd
from concourse.masks import make_identity

F32 = mybir.dt.float32
BF16 = mybir.dt.bfloat16
I32 = mybir.dt.int32
AF = mybir.ActivationFunctionType
ALU = mybir.AluOpType
AX = mybir.AxisListType

# ---- problem constants ----
B, T, S = 8, 16, 64
NPT, NL = 24, 16
EPT = 48
V, H, NH, FF = 30522, 768, 12, 3072
R, NB, H1, H2 = 16, 30, 256, 256
CLF, LBL = 768, 7
DH = H // NH

NCORES = 8
TPC = T                      # turns per core
UT_TOK = TPC * S             # 1024
ND_TOK = TPC * NPT * NL      # 6144
NNODES = TPC * NPT           # 384
NEDGES = TPC * EPT           # 768
UT_TILES = UT_TOK // 128     # 8
ND_TILES = ND_TOK // 128     # 48
NODE_TILES = NNODES // 128   # 3
EDGE_TILES = NEDGES // 128   # 6


def _bf(x):
    return np.ascontiguousarray(
        np.asarray(x, dtype=np.float32)).astype(ml_dtypes.bfloat16)


# ======================================================================
# Graph builder
# ======================================================================

def _build_graph(phases=("u", "n", "r", "c"), probe=None, ncores=NCORES):
    nc = bacc.Bacc("TRN2", target_bir_lowering=False, debug=False,
                   num_devices=ncores)

    def din(name, shape, dt=BF16):
        return nc.dram_tensor(name, shape, dt, kind="ExternalInput").ap()

    d = {}
    d["utt_ids"] = din("utt_ids", [UT_TOK, 1], I32)
    d["node_ids"] = din("node_ids", [ND_TOK, 1], I32)
    d["emb_u"] = din("emb_u", [V, H])
    d["emb_n"] = din("emb_n", [V, H])
    d["pos_u"] = din("pos_u", [128, H])
    d["pos_n"] = din("pos_n", [128, H])
    d["mask_u"] = din("mask_u", [128, 128], F32)
    d["mask_n"] = din("mask_n", [128, 128], F32)
    for enc in ("u", "n"):
        for w in ("wq", "wk", "wv", "wo"):
            d[w + enc] = din(w + enc, [H, H])
        d["w1" + enc] = din("w1" + enc, [H, FF])
        d["w2" + enc] = din("w2" + enc, [FF, H])
        for bn, dim in [("bq", H), ("bk", H), ("bv", H), ("bo", H),
                        ("bf1", FF), ("bf2", H)]:
            d[bn + enc] = din(bn + enc, [1, dim])
        for gn in ("g0", "b0", "g1", "b1"):
            d[gn + enc] = din(gn + enc, [1, H])
    d["g2u"] = din("g2u", [1, H])
    d["b2u"] = din("b2u", [1, H])
    d["g2n"] = din("g2n", [1, H])
    d["b2n"] = din("b2n", [1, H])
    d["poolw"] = din("poolw", [1, H])
    d["bm8"] = din("bm8", [128, 8])
    d["W1"] = din("W1", [R * H, H1])
    d["W2"] = din("W2", [R * H1, H2])
    d["U1"] = din("U1", [H, R])
    d["V1"] = din("V1", [H, R])
    d["U2"] = din("U2", [H1, R])
    d["V2"] = din("V2", [H1, R])
    d["AdstNE"] = din("AdstNE", [NNODES, NEDGES])
    d["AsrcNE"] = din("AsrcNE", [NNODES, NEDGES])
    d["AdstEN"] = din("AdstEN", [NEDGES, NNODES])
    d["AsrcR"] = din("AsrcR", [R * NNODES, NEDGES])
    d["T1hot"] = din("T1hot", [NEDGES, R])
    d["w1c"] = din("w1c", [H + H2, CLF])
    d["w2c"] = din("w2c", [CLF, CLF])
    d["w3c"] = din("w3c", [CLF, LBL])
    d["b1c"] = din("b1c", [1, CLF])
    d["b2c"] = din("b2c", [1, CLF])
    d["b3c"] = din("b3c", [1, LBL])
    d["lblhot"] = din("lblhot", [TPC, LBL], F32)
    d["meanT"] = din("meanT", [NNODES, TPC])

    out = nc.dram_tensor("out", [TPC, 8], F32, kind="ExternalOutput").ap()
    dbg = None
    if probe is not None:
        dbg = nc.dram_tensor("dbg", [128, 3 * H], F32,
                             kind="ExternalOutput").ap()

    with tile.TileContext(nc) as tc:
        _emit(nc, tc, d, out, phases=phases, probe=probe, dbg=dbg)
    nc.compile()
    return nc


def _emit(nc, tc, d, out, phases=("u", "n", "r", "c"), probe=None, dbg=None):
    ctx = contextlib.ExitStack()
    with ctx:
        glob = ctx.enter_context(tc.tile_pool(name="glob", bufs=1))
        small = ctx.enter_context(tc.tile_pool(name="small", bufs=4))

        ones1 = glob.tile([1, 128], BF16)
        nc.vector.memset(ones1[:], 1.0)
        epssb = glob.tile([128, 1], F32)
        nc.vector.memset(epssb[:], 1e-12)
        id32 = glob.tile([128, 128], F32)
        make_identity(nc, id32[:])

        utt_cls = glob.tile([16, H], BF16)
        uttf = glob.tile([16, H], BF16)
        x0sb = glob.tile([128, NODE_TILES, H], BF16)
        x1sb = glob.tile([128, NODE_TILES, H1], BF16)
        x2sb = glob.tile([128, NODE_TILES, H2], BF16)
        bm8 = glob.tile([128, 8], BF16)
        nc.sync.dma_start(bm8[:], d["bm8"][:])

        def layernorm(x_in, x_out, g=None, b=None, p=128):
            fs = x_in.shape[-1]
            nsub = fs // 256
            stats = small.tile([p, nsub, 6], F32, tag="lnstats")
            for i in range(nsub):
                nc.vector.bn_stats(stats[:, i, :], x_in[:, ts(i, 256)])
            mv = small.tile([p, 2], F32, tag="lnmv")
            nc.vector.bn_aggr(mv[:], stats[:])
            rstd = small.tile([p, 1], F32, tag="lnrstd")
            nc.scalar.activation(rstd[:], mv[:, 1:2], AF.Sqrt, bias=epssb[:p, :])
            nc.vector.reciprocal(rstd[:], rstd[:])
            nc.vector.tensor_scalar(x_out[:], x_in[:], mv[:, 0:1], rstd[:],
                                    op0=ALU.subtract, op1=ALU.mult)
            if g is not None:
                nc.vector.tensor_mul(x_out[:], x_out[:], g[:p, :])
                nc.vector.tensor_add(x_out[:], x_out[:], b[:p, :])

        def proj(pp, xT, w_sb, b_sb, out_sb, nck, n_out, act=None, p=128):
            for g0 in range(0, n_out, 512):
                gw = min(512, n_out - g0)
                ps = pp.tile([p, 512], F32, tag="proj")
                nc.tensor.matmul(ps[:, :gw], ones1[:, :p], b_sb[:, g0:g0 + gw],
                                 start=True, stop=False)
                for c in range(nck):
                    nc.tensor.matmul(ps[:, :gw], xT[:, c, :p],
                                     w_sb[:, c, g0:g0 + gw],
                                     start=False, stop=(c == nck - 1))
                nc.scalar.activation(out_sb[:, g0:g0 + gw], ps[:, :gw],
                                     act if act is not None else AF.Copy)

        def proj_res(pp, xT, w_sb, b_sb, x_res, out_sb, nck, p=128):
            # out = x @ W + b + x_res  (residual read from PSUM by DVE)
            for g0, gw in ((0, 512), (512, 256)):
                ps = pp.tile([p, 512], F32, tag="proj")
                nc.tensor.matmul(ps[:, :gw], ones1[:, :p], b_sb[:, g0:g0 + gw],
                                 start=True, stop=False)
                for c in range(nck):
                    nc.tensor.matmul(ps[:, :gw], xT[:, c, :p],
                                     w_sb[:, c, g0:g0 + gw],
                                     start=False, stop=(c == nck - 1))
                nc.vector.tensor_add(out_sb[:, g0:g0 + gw], ps[:, :gw],
                                     x_res[:, g0:g0 + gw])

        # ================= encoder phase =================
        with (
            tc.tile_pool(name="encw", bufs=1) as encw,
            tc.tile_pool(name="workA", bufs=2) as workA,
            tc.tile_pool(name="workB", bufs=1) as workB,
            tc.tile_pool(name="ffnw", bufs=2) as ffnw,
            tc.tile_pool(name="attw", bufs=3) as attw,
            tc.tile_pool(name="epsum", bufs=2, space="PSUM") as pp,
            tc.tile_pool(name="epsum_s", bufs=2, space="PSUM") as pps,
            tc.tile_pool(name="epsum1", bufs=1, space="PSUM") as pp1,
        ):
            def ffn(xT, w1sb, bf1sb, w2sb, bf2sb, x_res, x2p_out, p=128):
                f2A = pp1.tile([p, 512], F32, tag="f2A")
                f2B = pp1.tile([p, 256], F32, tag="f2B")
                nc.tensor.matmul(f2A[:], ones1[:, :p], bf2sb[:, 0:512],
                                 start=True, stop=False)
                nc.tensor.matmul(f2B[:], ones1[:, :p], bf2sb[:, 512:H],
                                 start=True, stop=False)
                for g in range(4):
                    h1g = ffnw.tile([p, 768], BF16, tag="h1g")
                    for sub, w in ((0, 512), (512, 256)):
                        ps = pp.tile([p, 512], F32, tag="proj")
                        col0 = 768 * g + sub
                        nc.tensor.matmul(ps[:, :w], ones1[:, :p],
                                         bf1sb[:, col0:col0 + w],
                                         start=True, stop=False)
                        for c in range(6):
                            nc.tensor.matmul(ps[:, :w], xT[:, c, :p],
                                             w1sb[:, c, col0:col0 + w],
                                             start=False, stop=(c == 5))
                        nc.scalar.activation(h1g[:, sub:sub + w], ps[:, :w],
                                             AF.Gelu_apprx_tanh)
                    h1gT = ffnw.tile([128, 6, p], BF16, tag="h1gT")
                    for c in range(6):
                        nc.sync.dma_start(h1gT[:, c, :], h1g[:, ts(c, 128)],
                                          transpose=True)
                    last = (g == 3)
                    for c in range(6):
                        gc = 6 * g + c
                        nc.tensor.matmul(f2A[:], h1gT[:, c, :p],
                                         w2sb[:, gc, 0:512],
                                         start=False, stop=(last and c == 5))
                        nc.tensor.matmul(f2B[:], h1gT[:, c, :p],
                                         w2sb[:, gc, 512:H],
                                         start=False, stop=(last and c == 5))
                nc.vector.tensor_add(x2p_out[:, 0:512], f2A[:], x_res[:, 0:512])
                nc.vector.tensor_add(x2p_out[:, 512:H], f2B[:], x_res[:, 512:H])

            def load_enc_weights(enc):
                wsb = {}
                for n in ("wq", "wk", "wv", "wo"):
                    t = encw.tile([128, 6, H], BF16, tag=n)
                    for c in range(6):
                        nc.sync.dma_start(t[:, c, :], d[n + enc][ts(c, 128), :])
                    wsb[n] = t
                t = encw.tile([128, 6, FF], BF16, tag="w1")
                for c in range(6):
                    nc.sync.dma_start(t[:, c, :], d["w1" + enc][ts(c, 128), :])
                wsb["w1"] = t
                t = encw.tile([128, 24, H], BF16, tag="w2")
                for c in range(24):
                    nc.sync.dma_start(t[:, c, :], d["w2" + enc][ts(c, 128), :])
                wsb["w2"] = t
                for bn, dim in [("bq", H), ("bk", H), ("bv", H), ("bo", H),
                                ("bf1", FF), ("bf2", H)]:
                    t = encw.tile([1, dim], BF16, tag=bn)
                    nc.sync.dma_start(t[:], d[bn + enc][:])
                    wsb[bn] = t
                for gn in ("g0", "b0", "g1", "b1"):
                    t = encw.tile([128, H], BF16, tag=gn)
                    nc.sync.dma_start(t[:], d[gn + enc].to_broadcast([128, H]))
                    wsb[gn] = t
                t = encw.tile([128, H], BF16, tag="pos")
                nc.sync.dma_start(t[:], d["pos_" + enc][:])
                wsb["pos"] = t
                t = encw.tile([128, 128], F32, tag="mask")
                nc.sync.dma_start(t[:], d["mask_" + enc][:])
                wsb["mask"] = t
                return wsb

            def transpose_ck(x_bf16, nck, tag):
                xT = workA.tile([128, nck, 128], BF16, tag=tag)
                for c in range(nck):
                    nc.sync.dma_start(xT[:, c, :], x_bf16[:, ts(c, 128)],
                                      transpose=True)
                return xT

            def enc_tile(wsb, ids_dram, emb_dram, t, is_node, tl=None,
                         pool_env=None):
                idx = small.tile([128, 1], I32, tag="idx")
                nc.sync.dma_start(idx[:], ids_dram[ts(t, 128), :])
                xg = workA.tile([128, H], BF16, tag="xg")
                nc.gpsimd.indirect_dma_start(
                    out=xg[:], out_offset=None, in_=emb_dram[:],
                    in_offset=bass.IndirectOffsetOnAxis(ap=idx[:, :1], axis=0))
                nc.vector.tensor_add(xg[:], xg[:], wsb["pos"][:])
                x0 = workA.tile([128, H], BF16, tag="x0")
                layernorm(xg, x0, wsb["g0"], wsb["b0"])

                x0T = transpose_ck(x0, 6, "xT6")
                q_sb = workB.tile([128, H], BF16, tag="qk_sb")
                proj(pp, x0T, wsb["wq"], wsb["bq"], q_sb, 6, H)
                qT = transpose_ck(q_sb, 6, "qT")
                k_sb = workB.tile([128, H], BF16, tag="qk_sb")
                proj(pp, x0T, wsb["wk"], wsb["bk"], k_sb, 6, H)
                kT = transpose_ck(k_sb, 6, "kT")
                v_sb = workB.tile([128, H], BF16, tag="v_sb")
                proj(pp, x0T, wsb["wv"], wsb["bv"], v_sb, 6, H)

                ctxT = workA.tile([128, 6, 128], BF16, tag="ctxT")
                for hp in range(6):
                    ctx_ps = pp1.tile([128, 128], F32, tag="ctx_ps")
                    for hh in range(2):
                        h = 2 * hp + hh
                        r0 = 64 * hh
                        s_ps = pps.tile([128, 128], F32, tag="s_ps")
                        nc.tensor.matmul(s_ps[:],
                                         qT[ds(64 * (h % 2), 64), h // 2, :],
                                         kT[ds(64 * (h % 2), 64), h // 2, :],
                                         start=True, stop=True)
                        nc.vector.tensor_add(s_ps[:], s_ps[:], wsb["mask"][:])
                        mx = small.tile([128, 1], F32, tag="mx")
                        nc.vector.reduce_max(mx[:], s_ps[:], axis=AX.X)
                        negm = small.tile([128, 1], F32, tag="negm")
                        nc.vector.tensor_scalar_mul(negm[:], mx[:], -0.125)
                        e_sb = attw.tile([128, 128], BF16, tag="e_sb")
                        den = small.tile([128, 1], F32, tag="den")
                        nc.scalar.activation(e_sb[:], s_ps[:], AF.Exp,
                                             bias=negm[:], scale=0.125,
                                             accum_out=den[:])
                        nc.vector.reciprocal(den[:], den[:])
                        a_sb = attw.tile([128, 128], BF16, tag="a_sb")
                        nc.vector.tensor_scalar_mul(a_sb[:], e_sb[:], den[:])
                        aT = attw.tile([128, 128], BF16, tag="aT")
                        nc.sync.dma_start(aT[:], a_sb[:], transpose=True)
                        nc.tensor.matmul(ctx_ps[ds(r0, 64), :],
                                         v_sb[:, ds(64 * h, 64)], aT[:],
                                         start=True, stop=True,
                                         tile_position=(0, r0))
                    nc.scalar.activation(ctxT[:, hp, :], ctx_ps[:], AF.Copy)

                x1p = workA.tile([128, H], BF16, tag="x1p")
                proj_res(pp, ctxT, wsb["wo"], wsb["bo"], x0, x1p, 6)

                if not is_node:
                    nc.sync.dma_start(utt_cls[2 * t:2 * t + 1, :], x1p[0:1, :])
                    nc.sync.dma_start(utt_cls[2 * t + 1:2 * t + 2, :],
                                      x1p[64:65, :])
                    return

                x1 = workB.tile([128, H], BF16, tag="x1")
                layernorm(x1p, x1, wsb["g1"], wsb["b1"])
                x1T = transpose_ck(x1, 6, "xT6")
                x2p = workA.tile([128, H], BF16, tag="x1p")
                ffn(x1T, wsb["w1"], wsb["bf1"], wsb["w2"], wsb["bf2"], x1, x2p)
                x2n = workB.tile([128, H], BF16, tag="x2n")
                layernorm(x2p, x2n)

                # attention pool (g2/b2 folded: poolw = g2*pool_w)
                poolw_b, g2n_b, b2n_b = pool_env
                poolmode = os.environ.get("KERNEL_POOLMODE", "full")
                if poolmode == "nopool":
                    stage8 = workB.tile([8, H], BF16, tag="stage8")
                    nc.scalar.activation(stage8[:], x2n[0:8, :], AF.Copy)
                    nl = t % 16
                    nc.sync.dma_start(x0sb[8 * nl:8 * nl + 8, t // 16, :],
                                      stage8[:])
                    return
                sc = small.tile([128, 1], F32, tag="poolsc")
                scratch = workA.tile([128, H], BF16, tag="xg")
                nc.vector.tensor_mul(scratch[:], x2n[:], poolw_b[:])
                nc.vector.reduce_sum(sc[:], scratch[:], axis=AX.X)
                poolT = pp1.tile([128, 128], F32, tag="ctx_ps")
                sp = poolT[0:1, :]
                nc.tensor.transpose(sp, sc[:, 0:1], id32[:])
                if poolmode == "sm1":
                    dummy = small.tile([1, 128], F32, tag="pes")
                    nc.vector.tensor_copy(dummy[:], sp)
                    stage8 = workB.tile([8, H], BF16, tag="stage8")
                    nc.scalar.activation(stage8[:], x2n[0:8, :], AF.Copy)
                    nl = t % 16
                    nc.sync.dma_start(x0sb[8 * nl:8 * nl + 8, t // 16, :],
                                      stage8[:])
                    return
                sp3 = sp.rearrange("a (n l) -> a n l", n=8)
                mx8 = small.tile([1, 8, 1], F32, tag="pmx")
                nc.vector.reduce_max(mx8[:], sp3, axis=AX.X)
                es = small.tile([1, 128], F32, tag="pes")
                es3 = es[:].rearrange("a (n l) -> a n l", n=8)
                nc.vector.tensor_tensor(out=es3, in0=sp3,
                                        in1=mx8[:].to_broadcast([1, 8, NL]),
                                        op=ALU.subtract)
                nc.scalar.activation(es[:], es[:], AF.Exp)
                sm8 = small.tile([1, 8, 1], F32, tag="psm")
                nc.vector.reduce_sum(sm8[:], es3, axis=AX.X)
                nc.vector.reciprocal(sm8[:], sm8[:])
                aw = small.tile([1, 128], F32, tag="paw")
                nc.vector.tensor_tensor(
                    out=aw[:].rearrange("a (n l) -> a n l", n=8), in0=es3,
                    in1=sm8[:].to_broadcast([1, 8, NL]), op=ALU.mult)
                if poolmode == "sm2":
                    stage8 = workB.tile([8, H], BF16, tag="stage8")
                    nc.scalar.activation(stage8[:], x2n[0:8, :], AF.Copy)
                    nl = t % 16
                    nc.sync.dma_start(x0sb[8 * nl:8 * nl + 8, t // 16, :],
                                      stage8[:])
                    return
                awT_t = pp1.tile([128, 128], F32, tag="ctx_ps")
                nc.tensor.transpose(awT_t[:, 0:1], aw[0:1, :], id32[0:1, 0:1])
                ablk = workB.tile([128, 8], BF16, tag="ablk")
                nc.vector.tensor_scalar_mul(ablk[:], bm8[:], awT_t[:, 0:1])
                if poolmode == "sm3":
                    stage8 = workB.tile([8, H], BF16, tag="stage8")
                    nc.scalar.activation(stage8[:], x2n[0:8, :], AF.Copy)
                    nl = t % 16
                    nc.sync.dma_start(x0sb[8 * nl:8 * nl + 8, t // 16, :],
                                      stage8[:])
                    return
                stage8 = workB.tile([8, H], BF16, tag="stage8")
                for g0, gw in ((0, 512), (512, 256)):
                    x0ps = pp.tile([8, 512], F32, tag="proj")
                    nc.tensor.matmul(x0ps[:, :gw], ablk[:],
                                     x2n[:, g0:g0 + gw], start=True, stop=True)
                    nc.vector.tensor_mul(stage8[:, g0:g0 + gw],
                                         x0ps[:, :gw], g2n_b[0:8, g0:g0 + gw])
                nc.vector.tensor_add(stage8[:], stage8[:], b2n_b[0:8, :])
                nl = t % 16
                nc.sync.dma_start(x0sb[8 * nl:8 * nl + 8, t // 16, :],
                                  stage8[:])

            # ---------- utterance encoder ----------
            if "u" in phases:
                wsb = load_enc_weights("u")
                for t in range(UT_TILES):
                    enc_tile(wsb, d["utt_ids"], d["emb_u"], t, is_node=False)

                g2u_b = encw.tile([128, H], BF16, tag="g2x")
                nc.sync.dma_start(g2u_b[:], d["g2u"].to_broadcast([128, H]))
                b2u_b = encw.tile([128, H], BF16, tag="b2x")
                nc.sync.dma_start(b2u_b[:], d["b2u"].to_broadcast([128, H]))
                x1c = workB.tile([16, H], BF16, tag="x1")
                layernorm(utt_cls, x1c, wsb["g1"], wsb["b1"], p=16)
                x1cT = workA.tile([128, 6, 16], BF16, tag="xT6")
                for c in range(6):
                    nc.sync.dma_start(x1cT[:, c, :], x1c[:, ts(c, 128)],
                                      transpose=True)
                x2pc = workA.tile([16, H], BF16, tag="x1p")
                ffn(x1cT, wsb["w1"], wsb["bf1"], wsb["w2"], wsb["bf2"], x1c,
                    x2pc, p=16)
                layernorm(x2pc, uttf, g2u_b, b2u_b, p=16)

            # ---------- node encoder ----------
            if "n" in phases:
                wsb = load_enc_weights("n")
                poolw_b = encw.tile([128, H], BF16, tag="poolw")
                nc.sync.dma_start(poolw_b[:],
                                  d["poolw"].to_broadcast([128, H]))
                g2n_b = encw.tile([128, H], BF16, tag="g2x")
                nc.sync.dma_start(g2n_b[:], d["g2n"].to_broadcast([128, H]))
                b2n_b = encw.tile([128, H], BF16, tag="b2x")
                nc.sync.dma_start(b2n_b[:], d["b2n"].to_broadcast([128, H]))

                env = (poolw_b, g2n_b, b2n_b)
                for t in range(ND_TILES):
                    enc_tile(wsb, d["node_ids"], d["emb_n"], t,
                             is_node=True, pool_env=env)

        # ================= RGAT phase =================
        if "r" not in phases:
            _probe_out(nc, probe, dbg, uttf=uttf, x0sb=x0sb, x1sb=x1sb,
                       x2sb=x2sb, utt_cls=utt_cls)
            return
        with (
            tc.tile_pool(name="rgw", bufs=1) as rgw,
            tc.tile_pool(name="rstream", bufs=4) as rstream,
            tc.tile_pool(name="rwork", bufs=2) as rwork,
            tc.tile_pool(name="rpsum2", bufs=2, space="PSUM") as rp2,
            tc.tile_pool(name="rpsum1", bufs=1, space="PSUM") as rp1,
        ):
            adst_ne = rgw.tile([128, NODE_TILES, NEDGES], BF16)
            asrc_ne = rgw.tile([128, NODE_TILES, NEDGES], BF16)
            adst_en = rgw.tile([128, EDGE_TILES, NNODES], BF16)
            t1hot = rgw.tile([128, EDGE_TILES, R], BF16)
            for ntb in range(NODE_TILES):
                nc.sync.dma_start(adst_ne[:, ntb, :],
                                  d["AdstNE"][ts(ntb, 128), :])
                nc.sync.dma_start(asrc_ne[:, ntb, :],
                                  d["AsrcNE"][ts(ntb, 128), :])
            for etb in range(EDGE_TILES):
                nc.sync.dma_start(adst_en[:, etb, :],
                                  d["AdstEN"][ts(etb, 128), :])
                nc.sync.dma_start(t1hot[:, etb, :],
                                  d["T1hot"][ts(etb, 128), :])

            def conv(x_in, nck, hin, hout, Wd, Ud, Vd, x_out):
                xT = rwork.tile([128, NODE_TILES, nck, 128], BF16, tag="rxT")
                for ntb in range(NODE_TILES):
                    for c in range(nck):
                        nc.sync.dma_start(xT[:, ntb, c, :],
                                          x_in[:, ntb, ts(c, 128)],
                                          transpose=True)
                uv = rstream.tile([128, nck, 2 * R], BF16, tag="ruv")
                for c in range(nck):
                    nc.sync.dma_start(uv[:, c, 0:R], Ud[ts(c, 128), :])
                    nc.sync.dma_start(uv[:, c, R:2 * R], Vd[ts(c, 128), :])
                psb = rwork.tile([128, NODE_TILES, 2 * R], BF16, tag="rpq")
                for ntb in range(NODE_TILES):
                    pq = rp1.tile([128, 2 * R], F32, tag="rpq_ps")
                    for c in range(nck):
                        nc.tensor.matmul(pq[:], xT[:, ntb, c, :], uv[:, c, :],
                                         start=(c == 0), stop=(c == nck - 1))
                    nc.scalar.activation(psb[:, ntb, :], pq[:], AF.Copy)
                ysb = rgw.tile([128, NODE_TILES, R, hout], BF16, tag="rY")
                for r in range(R):
                    wrt = rstream.tile([128, nck, hout], BF16, tag="rwr")
                    for c in range(nck):
                        nc.sync.dma_start(wrt[:, c, :],
                                          Wd[ds(r * hin + c * 128, 128), :])
                    for ntb in range(NODE_TILES):
                        yp = rp2.tile([128, hout], F32, tag="ry_ps")
                        for c in range(nck):
                            nc.tensor.matmul(yp[:], xT[:, ntb, c, :],
                                             wrt[:, c, :], start=(c == 0),
                                             stop=(c == nck - 1))
                        nc.scalar.activation(ysb[:, ntb, r, :], yp[:], AF.Copy)
                expE = rwork.tile([128, EDGE_TILES], F32, tag="rexpE")
                for etb in range(EDGE_TILES):
                    eps = rp1.tile([128, R], F32, tag="re_ps")
                    for ntb in range(NODE_TILES):
                        nc.tensor.matmul(eps[:],
                                         adst_ne[:, ntb, ts(etb, 128)],
                                         psb[:, ntb, 0:R],
                                         start=(ntb == 0), stop=False)
                    for ntb in range(NODE_TILES):
                        nc.tensor.matmul(eps[:],
                                         asrc_ne[:, ntb, ts(etb, 128)],
                                         psb[:, ntb, R:2 * R],
                                         start=False, stop=(ntb == 2))
                    scr = rwork.tile([128, R], BF16, tag="rescr")
                    eraw = small.tile([128, 1], F32, tag="reraw")
                    nc.vector.tensor_mul(scr[:], eps[:], t1hot[:, etb, :])
                    nc.vector.reduce_sum(eraw[:], scr[:], axis=AX.X)
                    nc.scalar.activation(eraw[:], eraw[:], AF.Lrelu, alpha=0.2)
                    nc.scalar.activation(expE[:, etb:etb + 1], eraw[:], AF.Exp)
                wmsg = rwork.tile([128, EDGE_TILES, hout + 1], BF16,
                                  tag="rwmsg")
                for etb in range(EDGE_TILES):
                    mps = rp2.tile([128, hout], F32, tag="rm_ps")
                    first = True
                    for r in range(R):
                        art = rstream.tile([128, NODE_TILES, 128], BF16,
                                           tag="rasrcr")
                        for ntb in range(NODE_TILES):
                            nc.sync.dma_start(
                                art[:, ntb, :],
                                d["AsrcR"][ds(r * NNODES + ntb * 128, 128),
                                           ts(etb, 128)])
                        for ntb in range(NODE_TILES):
                            nc.tensor.matmul(mps[:], art[:, ntb, :],
                                             ysb[:, ntb, r, :], start=first,
                                             stop=(r == R - 1 and ntb == 2))
                            first = False
                    nc.vector.tensor_scalar_mul(wmsg[:, etb, 0:hout], mps[:],
                                                expE[:, etb:etb + 1])
                    nc.vector.tensor_copy(wmsg[:, etb, hout:hout + 1],
                                          expE[:, etb:etb + 1])
                for ntb in range(NODE_TILES):
                    aps = rp1.tile([128, hout + 1], F32, tag="ragg_ps")
                    for etb in range(EDGE_TILES):
                        nc.tensor.matmul(aps[:],
                                         adst_en[:, etb, ts(ntb, 128)],
                                         wmsg[:, etb, :],
                                         start=(etb == 0),
                                         stop=(etb == EDGE_TILES - 1))
                    rden = small.tile([128, 1], F32, tag="rden")
                    # isolated nodes have den=0; clamp so agg(0)*recip -> 0
                    nc.vector.tensor_scalar_max(rden[:],
                                                aps[:, hout:hout + 1], 1e-30)
                    nc.vector.reciprocal(rden[:], rden[:])
                    nc.vector.tensor_scalar_mul(x_out[:, ntb, :],
                                                aps[:, 0:hout], rden[:])

            conv(x0sb, 6, H, H1, d["W1"], d["U1"], d["V1"], x1sb)
            conv(x1sb, 2, H1, H2, d["W2"], d["U2"], d["V2"], x2sb)

        # ================= classifier phase =================
        if "c" not in phases:
            _probe_out(nc, probe, dbg, uttf=uttf, x0sb=x0sb, x1sb=x1sb,
                       x2sb=x2sb, utt_cls=utt_cls)
            return
        with (
            tc.tile_pool(name="cw", bufs=1) as cw,
            tc.tile_pool(name="cpsum", bufs=2, space="PSUM") as cp,
        ):
            meanT = cw.tile([128, NODE_TILES, TPC], BF16)
            for ntb in range(NODE_TILES):
                nc.sync.dma_start(meanT[:, ntb, :],
                                  d["meanT"][ts(ntb, 128), :])
            ip = cp.tile([16, H2], F32, tag="inner_ps")
            for ntb in range(NODE_TILES):
                nc.tensor.matmul(ip[:], meanT[:, ntb, :], x2sb[:, ntb, :],
                                 start=(ntb == 0), stop=(ntb == NODE_TILES - 1))
            inner = cw.tile([16, H2], BF16)
            nc.scalar.activation(inner[:], ip[:], AF.Copy)

            featT = cw.tile([128, 8, 16], BF16)
            for c in range(6):
                nc.sync.dma_start(featT[:, c, :], uttf[:, ts(c, 128)],
                                  transpose=True)
            for c in range(2):
                nc.sync.dma_start(featT[:, 6 + c, :], inner[:, ts(c, 128)],
                                  transpose=True)

            w1c = cw.tile([128, 8, CLF], BF16)
            for c in range(8):
                nc.sync.dma_start(w1c[:, c, :], d["w1c"][ts(c, 128), :])
            w2c = cw.tile([128, 6, CLF], BF16)
            for c in range(6):
                nc.sync.dma_start(w2c[:, c, :], d["w2c"][ts(c, 128), :])
            w3c = cw.tile([128, 6, LBL], BF16)
            for c in range(6):
                nc.sync.dma_start(w3c[:, c, :], d["w3c"][ts(c, 128), :])
            bcs = {}
            for bn, dim in [("b1c", CLF), ("b2c", CLF), ("b3c", LBL)]:
                t = cw.tile([1, dim], BF16, tag=bn)
                nc.sync.dma_start(t[:], d[bn][:])
                bcs[bn] = t
            lblh = cw.tile([16, LBL], F32)
            nc.sync.dma_start(lblh[:], d["lblhot"][:])

            z1 = cw.tile([16, CLF], BF16)
            proj(cp, featT, w1c, bcs["b1c"], z1, 8, CLF, act=AF.Relu, p=16)
            z1T = cw.tile([128, 6, 16], BF16)
            for c in range(6):
                nc.sync.dma_start(z1T[:, c, :], z1[:, ts(c, 128)],
                                  transpose=True)
            z2 = cw.tile([16, CLF], BF16)
            proj(cp, z1T, w2c, bcs["b2c"], z2, 6, CLF, act=AF.Relu, p=16)
            z2T = cw.tile([128, 6, 16], BF16)
            for c in range(6):
                nc.sync.dma_start(z2T[:, c, :], z2[:, ts(c, 128)],
                                  transpose=True)
            lp = cp.tile([16, LBL], F32, tag="logit_ps")
            nc.tensor.matmul(lp[:], ones1[:, :16], bcs["b3c"][:],
                             start=True, stop=False)
            for c in range(6):
                nc.tensor.matmul(lp[:], z2T[:, c, :], w3c[:, c, :],
                                 start=False, stop=(c == 5))

            osb = cw.tile([16, 8], F32)
            nc.scalar.activation(osb[:, 0:LBL], lp[:], AF.Copy)
            mxl = small.tile([16, 1], F32, tag="cmx")
            nc.vector.reduce_max(mxl[:], lp[:], axis=AX.X)
            negml = small.tile([16, 1], F32, tag="cnegm")
            nc.vector.tensor_scalar_mul(negml[:], mxl[:], -1.0)
            el = cw.tile([16, LBL], F32)
            denl = small.tile([16, 1], F32, tag="cden")
            nc.scalar.activation(el[:], lp[:], AF.Exp, bias=negml[:],
                                 accum_out=denl[:])
            lsd = small.tile([16, 1], F32, tag="clsd")
            nc.scalar.activation(lsd[:], denl[:], AF.Ln)
            nc.vector.tensor_add(lsd[:], lsd[:], mxl[:])
            pick = small.tile([16, 1], F32, tag="cpick")
            pscr = cw.tile([16, LBL], F32)
            nc.vector.tensor_mul(pscr[:], lp[:], lblh[:])
            nc.vector.reduce_sum(pick[:], pscr[:], axis=AX.X)
            nc.vector.tensor_sub(osb[:, 7:8], lsd[:], pick[:])
            nc.sync.dma_start(out[:], osb[:])
        _probe_out(nc, probe, dbg, uttf=uttf, x0sb=x0sb, x1sb=x1sb,
                   x2sb=x2sb, utt_cls=utt_cls)


def _probe_out(nc, probe, dbg, **tiles):
    if probe is None:
        return
    t = tiles[probe]
    if probe in ("uttf", "utt_cls"):
        nc.gpsimd.dma_start(dbg[0:16, 0:H], t[:])
    elif probe == "x0sb":
        nc.gpsimd.dma_start(dbg[:], t[:].rearrange("p a b -> p (a b)"))
    else:  # x1sb / x2sb [128, 3, 256]
        nc.gpsimd.dma_start(dbg[:, 0:3 * 256],
                            t[:].rearrange("p a b -> p (a b)"))


# ======================================================================
# Host-side preparation
# ======================================================================

def _prep_shared(params):
    p = {k: {kk: np.asarray(vv, np.float32) for kk, vv in v.items()}
         if isinstance(v, dict) else np.asarray(v, np.float32)
         for k, v in params.items()}
    sh = {}
    for enc, key in (("u", "utt"), ("n", "node")):
        e = p[key]
        sh["emb_" + enc] = _bf(e["emb"])
        seqlen = S if enc == "u" else NL
        sh["pos_" + enc] = _bf(np.tile(e["pos"][:seqlen], (128 // seqlen, 1)))
        for w in ("wq", "wk", "wv", "wo"):
            sh[w + enc] = _bf(e[w])
        sh["w1" + enc] = _bf(e["w1"])
        sh["w2" + enc] = _bf(e["w2"])
        for bn in ("bq", "bk", "bv", "bo", "bf1", "bf2"):
            sh[bn + enc] = _bf(e[bn][None])
        for gn, src in (("g0", "ln0_g"), ("b0", "ln0_b"), ("g1", "ln1_g"),
                        ("b1", "ln1_b")):
            sh[gn + enc] = _bf(e[src][None])
    sh["g2u"] = _bf(p["utt"]["ln2_g"][None])
    sh["b2u"] = _bf(p["utt"]["ln2_b"][None])
    sh["g2n"] = _bf(p["node"]["ln2_g"][None])
    sh["b2n"] = _bf(p["node"]["ln2_b"][None])
    sh["poolw"] = _bf((p["node"]["ln2_g"] * p["pool_w"][:, 0])[None])

    def blockmask(bs):
        m = np.full((128, 128), -1e9, np.float32)
        for s0 in range(0, 128, bs):
            m[s0:s0 + bs, s0:s0 + bs] = 0.0
        return m
    sh["mask_u"] = blockmask(S)
    sh["mask_n"] = blockmask(NL)
    bm8 = np.zeros((128, 8), np.float32)
    for t in range(128):
        bm8[t, t // NL] = 1.0
    sh["bm8"] = _bf(bm8)

    for i, (cv, hin) in enumerate((("conv1", H), ("conv2", H1))):
        c = p[cv]
        W = np.einsum("rb,bio->rio", c["comp"], c["basis"]).astype(np.float32)
        sh[f"W{i + 1}"] = _bf(W.reshape(R * hin, -1))
        sh[f"U{i + 1}"] = _bf(np.einsum("rio,o->ir", W, c["aq"]))
        sh[f"V{i + 1}"] = _bf(np.einsum("rio,o->ir", W, c["ak"]))

    sh["w1c"] = _bf(p["clf_w1"])
    sh["w2c"] = _bf(p["clf_w2"])
    sh["w3c"] = _bf(p["clf_w3"])
    sh["b1c"] = _bf(p["clf_b1"][None])
    sh["b2c"] = _bf(p["clf_b2"][None])
    sh["b3c"] = _bf(p["clf_b3"][None])

    meanT = np.zeros((NNODES, TPC), np.float32)
    for n in range(NNODES):
        meanT[n, n // NPT] = 1.0 / NPT
    sh["meanT"] = _bf(meanT)
    return sh


def _prep_core(c, input_ids, node_input_ids, edge_index, edge_types, labels):
    m = {}
    m["utt_ids"] = np.asarray(input_ids[c], np.int32).reshape(UT_TOK, 1)
    m["node_ids"] = np.asarray(node_input_ids[c], np.int32).reshape(ND_TOK, 1)

    ei = np.asarray(edge_index[c], np.int64)
    et = np.asarray(edge_types[c], np.int64)
    src = (ei[..., 0] + np.arange(TPC)[:, None] * NPT).reshape(-1)
    dst = (ei[..., 1] + np.arange(TPC)[:, None] * NPT).reshape(-1)
    etf = et.reshape(-1)
    eids = np.arange(NEDGES)
    adst = np.zeros((NNODES, NEDGES), np.float32)
    adst[dst, eids] = 1.0
    asrc = np.zeros((NNODES, NEDGES), np.float32)
    asrc[src, eids] = 1.0
    m["AdstNE"] = _bf(adst)
    m["AsrcNE"] = _bf(asrc)
    m["AdstEN"] = _bf(adst.T)
    asrcr = np.zeros((R, NNODES, NEDGES), np.float32)
    asrcr[etf, src, eids] = 1.0
    m["AsrcR"] = _bf(asrcr.reshape(R * NNODES, NEDGES))
    t1 = np.zeros((NEDGES, R), np.float32)
    t1[eids, etf] = 1.0
    m["T1hot"] = _bf(t1)

    lbl = np.asarray(labels[c], np.int64)
    lh = np.zeros((TPC, LBL), np.float32)
    lh[np.arange(TPC), lbl] = 1.0
    m["lblhot"] = lh
    return m


_NC_CACHE = None


def _get_nc():
    global _NC_CACHE
    if _NC_CACHE is None:
        _NC_CACHE = _build_graph()
    return _NC_CACHE


def kernel(input_ids, attention_mask, node_input_ids, node_attention_mask,
           edge_index, edge_types, labels, params):
    nc = _get_nc()
    sh = _prep_shared(params)
    input_ids = np.asarray(input_ids)
    node_input_ids = np.asarray(node_input_ids)
    edge_index = np.asarray(edge_index)
    edge_types = np.asarray(edge_types)
    labels = np.asarray(labels)

    in_maps = []
    for c in range(NCORES):
        m = dict(sh)
        m.update(_prep_core(c, input_ids, node_input_ids, edge_index,
                            edge_types, labels))
        in_maps.append(m)

    trace = bool(int(os.environ.get("KERNEL_TRACE", "0")))
    res = run_bass_kernel_spmd(nc, in_maps, core_ids=list(range(NCORES)),
                               trace=trace)
    if trace:
        kernel.last_exec_time_ns = res.exec_time_ns

    outs = [res.results[c]["out"] for c in range(NCORES)]
    logits = np.concatenate([o[:, :LBL] for o in outs], axis=0)
    loss = np.float32(np.mean(np.concatenate([o[:, 7] for o in outs])))
    return logits.astype(np.float32), loss


# revision 23
# speedup vs baseline: 1.4439x; 1.4439x over previous
"""Self-contained Trainium2 Bass kernel for nn_AMRModel_48455821033670.

Data-parallel over B*T across 8 NeuronCores (1 dialog per core).
Per core: BERT-layer utterance encoder (1024 tok, CLS-only FFN tail),
BERT-layer node encoder (6144 tok) with fused attention pool, 2 RGAT
layers (dense per-relation matmuls with host-built incidence matrices),
MLP classifier + NLL loss. Token-major bf16 activations with
feature-major Q/K/h1 produced directly on the PE (no transposes);
batched DMA transposes feed the remaining lhsT operands.
"""

import contextlib
import os

import numpy as np
import ml_dtypes

import concourse.bass as bass
import concourse.bacc as bacc
import concourse.mybir as mybir
import concourse.tile as tile
from concourse.bass import ds, ts
from concourse.bass_utils import run_bass_kernel_spmd
from concourse.masks import make_identity

F32 = mybir.dt.float32
BF16 = mybir.dt.bfloat16
I32 = mybir.dt.int32
AF = mybir.ActivationFunctionType
ALU = mybir.AluOpType
AX = mybir.AxisListType

# ---- problem constants ----
B, T, S = 8, 16, 64
NPT, NL = 24, 16
EPT = 48
V, H, NH, FF = 30522, 768, 12, 3072
R, NB, H1, H2 = 16, 30, 256, 256
CLF, LBL = 768, 7
DH = H // NH

NCORES = 8
TPC = T
UT_TOK = TPC * S             # 1024
ND_TOK = TPC * NPT * NL      # 6144
NNODES = TPC * NPT           # 384
NEDGES = TPC * EPT           # 768
UT_TILES = UT_TOK // 128     # 8
ND_TILES = ND_TOK // 128     # 48
NODE_TILES = NNODES // 128   # 3
EDGE_TILES = NEDGES // 128   # 6


def _bf(x):
    return np.ascontiguousarray(
        np.asarray(x, dtype=np.float32)).astype(ml_dtypes.bfloat16)


# ======================================================================
# Graph builder
# ======================================================================

def _build_graph(phases=("u", "n", "r", "c"), probe=None, ncores=NCORES):
    nc = bacc.Bacc("TRN2", target_bir_lowering=False, debug=False,
                   num_devices=ncores)

    def din(name, shape, dt=BF16):
        return nc.dram_tensor(name, shape, dt, kind="ExternalInput").ap()

    d = {}
    d["utt_ids"] = din("utt_ids", [UT_TOK, 1], I32)
    d["node_ids"] = din("node_ids", [ND_TOK, 1], I32)
    d["emb_u"] = din("emb_u", [V, H])
    d["emb_n"] = din("emb_n", [V, H])
    d["pos_u"] = din("pos_u", [128, H])
    d["pos_n"] = din("pos_n", [128, H])
    d["mask_u"] = din("mask_u", [128, 128], F32)
    d["mask_n"] = din("mask_n", [128, 128], F32)
    for enc in ("u", "n"):
        for w in ("wq", "wk", "wv", "wo"):
            d[w + enc] = din(w + enc, [H, H])
        d["w1" + enc] = din("w1" + enc, [H, FF])
        d["w2" + enc] = din("w2" + enc, [FF, H])
        d["bqf" + enc] = din("bqf" + enc, [128, 6], F32)
        d["bkf" + enc] = din("bkf" + enc, [128, 6], F32)
        d["bf1f" + enc] = din("bf1f" + enc, [128, 24], F32)
        for bn, dim in [("bv", H), ("bo", H), ("bf2", H)]:
            d[bn + enc] = din(bn + enc, [1, dim])
        for gn in ("g0", "b0", "g1", "b1"):
            d[gn + enc] = din(gn + enc, [1, H])
    d["g2u"] = din("g2u", [1, H])
    d["b2u"] = din("b2u", [1, H])
    d["g2n"] = din("g2n", [1, H])
    d["b2n"] = din("b2n", [1, H])
    d["poolw"] = din("poolw", [1, H])
    d["bm8"] = din("bm8", [128, 8])
    d["W1"] = din("W1", [R * H, H1])
    d["W2"] = din("W2", [R * H1, H2])
    d["U1"] = din("U1", [H, R])
    d["V1"] = din("V1", [H, R])
    d["U2"] = din("U2", [H1, R])
    d["V2"] = din("V2", [H1, R])
    d["AdstNE"] = din("AdstNE", [NNODES, NEDGES])
    d["AsrcNE"] = din("AsrcNE", [NNODES, NEDGES])
    d["AdstEN"] = din("AdstEN", [NEDGES, NNODES])
    d["AsrcR"] = din("AsrcR", [R * NNODES, NEDGES])
    d["T1hot"] = din("T1hot", [NEDGES, R])
    d["w1c"] = din("w1c", [H + H2, CLF])
    d["w2c"] = din("w2c", [CLF, CLF])
    d["w3c"] = din("w3c", [CLF, LBL])
    d["b1c"] = din("b1c", [1, CLF])
    d["b2c"] = din("b2c", [1, CLF])
    d["b3c"] = din("b3c", [1, LBL])
    d["lblhot"] = din("lblhot", [TPC, LBL], F32)
    d["meanT"] = din("meanT", [NNODES, TPC])

    out = nc.dram_tensor("out", [TPC, 8], F32, kind="ExternalOutput").ap()
    dbg = None
    if probe is not None:
        dbg = nc.dram_tensor("dbg", [128, 3 * H], F32,
                             kind="ExternalOutput").ap()

    with tile.TileContext(nc) as tc:
        _emit(nc, tc, d, out, phases=phases, probe=probe, dbg=dbg)
    nc.compile()
    return nc


def _emit(nc, tc, d, out, phases=("u", "n", "r", "c"), probe=None, dbg=None):
    ctx = contextlib.ExitStack()
    with ctx:
        glob = ctx.enter_context(tc.tile_pool(name="glob", bufs=1))
        small = ctx.enter_context(tc.tile_pool(name="small", bufs=4))

        ones1 = glob.tile([1, 128], BF16)
        nc.vector.memset(ones1[:], 1.0)
        epssb = glob.tile([128, 1], F32)
        nc.vector.memset(epssb[:], 1e-12)
        id32 = glob.tile([128, 128], F32)
        make_identity(nc, id32[:])

        utt_cls = glob.tile([16, H], BF16)
        uttf = glob.tile([16, H], BF16)
        x0sb = glob.tile([128, NODE_TILES, H], BF16)
        x1sb = glob.tile([128, NODE_TILES, H1], BF16)
        x2sb = glob.tile([128, NODE_TILES, H2], BF16)
        bm8 = glob.tile([128, 8], BF16)
        nc.sync.dma_start(bm8[:], d["bm8"][:])

        def layernorm(x_in, x_out, g=None, b=None, p=128):
            fs = x_in.shape[-1]
            nsub = fs // 256
            stats = small.tile([p, nsub, 6], F32, tag="lnstats")
            for i in range(nsub):
                nc.vector.bn_stats(stats[:, i, :], x_in[:, ts(i, 256)])
            mv = small.tile([p, 2], F32, tag="lnmv")
            nc.vector.bn_aggr(mv[:], stats[:])
            rstd = small.tile([p, 1], F32, tag="lnrstd")
            nc.scalar.activation(rstd[:], mv[:, 1:2], AF.Sqrt,
                                 bias=epssb[:p, :])
            nc.vector.reciprocal(rstd[:], rstd[:])
            nc.vector.tensor_scalar(x_out[:], x_in[:], mv[:, 0:1], rstd[:],
                                    op0=ALU.subtract, op1=ALU.mult)
            if g is not None:
                nc.vector.tensor_mul(x_out[:], x_out[:], g[:p, :])
                nc.vector.tensor_add(x_out[:], x_out[:], b[:p, :])

        def proj(pp, xT, w_sb, b_sb, out_sb, nck, n_out, act=None, p=128,
                 out_via_dve=False):
            # token-major out [p, n_out] = x @ W + b
            for g0 in range(0, n_out, 512):
                gw = min(512, n_out - g0)
                ps = pp.tile([p, 512], F32, tag="proj")
                nc.tensor.matmul(ps[:, :gw], ones1[:, :p], b_sb[:, g0:g0 + gw],
                                 start=True, stop=False)
                for c in range(nck):
                    nc.tensor.matmul(ps[:, :gw], xT[:, c, :p],
                                     w_sb[:, c, g0:g0 + gw],
                                     start=False, stop=(c == nck - 1))
                if out_via_dve:
                    nc.vector.tensor_copy(out_sb[:, g0:g0 + gw], ps[:, :gw])
                else:
                    nc.scalar.activation(out_sb[:, g0:g0 + gw], ps[:, :gw],
                                         act if act is not None else AF.Copy)

        def proj_res(pp, xT, w_sb, b_sb, x_res, out_sb, nck, p=128):
            for g0, gw in ((0, 512), (512, 256)):
                ps = pp.tile([p, 512], F32, tag="proj")
                nc.tensor.matmul(ps[:, :gw], ones1[:, :p], b_sb[:, g0:g0 + gw],
                                 start=True, stop=False)
                for c in range(nck):
                    nc.tensor.matmul(ps[:, :gw], xT[:, c, :p],
                                     w_sb[:, c, g0:g0 + gw],
                                     start=False, stop=(c == nck - 1))
                nc.vector.tensor_add(out_sb[:, g0:g0 + gw], ps[:, :gw],
                                     x_res[:, g0:g0 + gw])

        # ================= encoder phase =================
        with (
            tc.tile_pool(name="encw", bufs=1) as encw,
            tc.tile_pool(name="workA", bufs=2) as workA,
            tc.tile_pool(name="workB", bufs=1) as workB,
            tc.tile_pool(name="ffnw", bufs=2) as ffnw,
            tc.tile_pool(name="attw", bufs=2) as attw,
            tc.tile_pool(name="epsum", bufs=2, space="PSUM") as pp,
            tc.tile_pool(name="epsum_f", bufs=3, space="PSUM") as ppf,
            tc.tile_pool(name="epsum1", bufs=1, space="PSUM") as pp1,
        ):
            def fm_proj(dst, w_sb, x_T, bias_fm, nck, p=128):
                # feature-major out: dst[:, c, :] = (x @ W + b)^T chunks
                for c in range(6):
                    fps = ppf.tile([128, 128], F32, tag="fm")
                    for ic in range(nck):
                        nc.tensor.matmul(fps[:, :p],
                                         w_sb[:, ic, ts(c, 128)],
                                         x_T[:, ic, :p],
                                         start=(ic == 0), stop=(ic == nck - 1))
                    nc.vector.tensor_scalar_add(dst[:, c, :p], fps[:, :p],
                                                bias_fm[:, c:c + 1])

            def ffn(xT, wsb, x_res, x2p_out, p=128):
                # h1 produced feature-major; w2 accumulated into f2A/f2B
                f2A = pp1.tile([p, 512], F32, tag="f2A")
                f2B = pp1.tile([p, 256], F32, tag="f2B")
                nc.tensor.matmul(f2A[:], ones1[:, :p], wsb["bf2"][:, 0:512],
                                 start=True, stop=False)
                nc.tensor.matmul(f2B[:], ones1[:, :p], wsb["bf2"][:, 512:H],
                                 start=True, stop=False)
                h1T = ffnw.tile([128, 24, p], BF16, tag="h1T")
                for c24 in range(24):
                    fps = ppf.tile([128, 128], F32, tag="fm")
                    for ic in range(6):
                        nc.tensor.matmul(fps[:, :p],
                                         wsb["w1"][:, ic, ts(c24, 128)],
                                         xT[:, ic, :p],
                                         start=(ic == 0), stop=(ic == 5))
                    nc.scalar.activation(h1T[:, c24, :p], fps[:, :p],
                                         AF.Gelu_apprx_tanh,
                                         bias=wsb["bf1f"][:, c24:c24 + 1])
                    nc.tensor.matmul(f2A[:], h1T[:, c24, :p],
                                     wsb["w2"][:, c24, 0:512],
                                     start=False, stop=(c24 == 23))
                    nc.tensor.matmul(f2B[:], h1T[:, c24, :p],
                                     wsb["w2"][:, c24, 512:H],
                                     start=False, stop=(c24 == 23))
                nc.vector.tensor_add(x2p_out[:, 0:512], f2A[:],
                                     x_res[:, 0:512])
                nc.vector.tensor_add(x2p_out[:, 512:H], f2B[:],
                                     x_res[:, 512:H])

            def load_enc_weights(enc):
                wsb = {}
                for n, nck in (("wq", 6), ("wk", 6), ("wv", 6), ("wo", 6)):
                    t = encw.tile([128, nck, H], BF16, tag=n)
                    nc.sync.dma_start(
                        t[:], d[n + enc].rearrange("(c p) f -> p c f", p=128))
                    wsb[n] = t
                t = encw.tile([128, 6, FF], BF16, tag="w1")
                nc.sync.dma_start(
                    t[:], d["w1" + enc].rearrange("(c p) f -> p c f", p=128))
                wsb["w1"] = t
                t = encw.tile([128, 24, H], BF16, tag="w2")
                nc.sync.dma_start(
                    t[:], d["w2" + enc].rearrange("(c p) f -> p c f", p=128))
                wsb["w2"] = t
                for bn, dim in [("bqf", 6), ("bkf", 6), ("bf1f", 24)]:
                    t = encw.tile([128, dim], F32, tag=bn)
                    nc.sync.dma_start(t[:], d[bn + enc][:])
                    wsb[bn] = t
                for bn, dim in [("bv", H), ("bo", H), ("bf2", H)]:
                    t = encw.tile([1, dim], BF16, tag=bn)
                    nc.sync.dma_start(t[:], d[bn + enc][:])
                    wsb[bn] = t
                for gn in ("g0", "b0", "g1", "b1"):
                    t = encw.tile([128, H], BF16, tag=gn)
                    nc.sync.dma_start(t[:], d[gn + enc].to_broadcast([128, H]))
                    wsb[gn] = t
                t = encw.tile([128, H], BF16, tag="pos")
                nc.sync.dma_start(t[:], d["pos_" + enc][:])
                wsb["pos"] = t
                t = encw.tile([128, 128], F32, tag="mask")
                nc.sync.dma_start(t[:], d["mask_" + enc][:])
                wsb["mask"] = t
                t = encw.tile([128, max(UT_TILES, ND_TILES)], I32, tag="idx")
                nt = UT_TILES if enc == "u" else ND_TILES
                nc.sync.dma_start(
                    t[:, 0:nt],
                    d[("utt_ids" if enc == "u" else "node_ids")].rearrange(
                        "(t p) o -> p (t o)", p=128))
                wsb["idx"] = t
                return wsb

            def transpose_ck(x_bf16, nck, tag, eng=None):
                xT = workA.tile([128, nck, 128], BF16, tag=tag)
                (eng or nc.sync).dma_start(xT[:], x_bf16[:], transpose=True)
                return xT

            def enc_tile(wsb, emb_dram, t, is_node, pool_env=None):
                xg = workA.tile([128, H], BF16, tag="xg")
                nc.gpsimd.indirect_dma_start(
                    out=xg[:], out_offset=None, in_=emb_dram[:],
                    in_offset=bass.IndirectOffsetOnAxis(
                        ap=wsb["idx"][:, t:t + 1], axis=0))
                nc.vector.tensor_add(xg[:], xg[:], wsb["pos"][:])
                x0 = workA.tile([128, H], BF16, tag="x0")
                layernorm(xg, x0, wsb["g0"], wsb["b0"])

                x0T = transpose_ck(x0, 6, "xT6", eng=nc.sync)
                qT = workA.tile([128, 6, 128], BF16, tag="qT")
                kT = workA.tile([128, 6, 128], BF16, tag="kT")
                fm_proj(qT, wsb["wq"], x0T, wsb["bqf"], 6)
                fm_proj(kT, wsb["wk"], x0T, wsb["bkf"], 6)
                v_sb = workB.tile([128, H], BF16, tag="v_sb")
                proj(pp, x0T, wsb["wv"], wsb["bv"], v_sb, 6, H,
                     out_via_dve=True)

                # attention: softmax all heads, then 2 batched A transposes
                a_all = attw.tile([128, 12, 128], BF16, tag="a_all")
                for h in range(12):
                    s_ps = ppf.tile([128, 128], F32, tag="fm")
                    nc.tensor.matmul(s_ps[:],
                                     qT[ds(64 * (h % 2), 64), h // 2, :],
                                     kT[ds(64 * (h % 2), 64), h // 2, :],
                                     start=True, stop=True)
                    nc.vector.tensor_add(s_ps[:], s_ps[:], wsb["mask"][:])
                    mx = small.tile([128, 1], F32, tag="mx")
                    nc.vector.reduce_max(mx[:], s_ps[:], axis=AX.X)
                    negm = small.tile([128, 1], F32, tag="negm")
                    nc.vector.tensor_scalar_mul(negm[:], mx[:], -0.125)
                    e_sb = attw.tile([128, 128], BF16, tag="e_sb")
                    den = small.tile([128, 1], F32, tag="den")
                    nc.scalar.activation(e_sb[:], s_ps[:], AF.Exp,
                                         bias=negm[:], scale=0.125,
                                         accum_out=den[:])
                    nc.vector.reciprocal(den[:], den[:])
                    nc.vector.tensor_scalar_mul(a_all[:, h, :], e_sb[:],
                                                den[:])
                aT_all = attw.tile([128, 12, 128], BF16, tag="aT_all")
                nc.sync.dma_start(aT_all[:, 0:6, :], a_all[:, 0:6, :],
                                  transpose=True)
                nc.scalar.dma_start(aT_all[:, 6:12, :], a_all[:, 6:12, :],
                                    transpose=True)

                ctxT = workA.tile([128, 6, 128], BF16, tag="ctxT")
                for hp in range(6):
                    ctx_ps = pp1.tile([128, 128], F32, tag="ctx_ps")
                    for hh in range(2):
                        h = 2 * hp + hh
                        r0 = 64 * hh
                        nc.tensor.matmul(ctx_ps[ds(r0, 64), :],
                                         v_sb[:, ds(64 * h, 64)],
                                         aT_all[:, h, :],
                                         start=True, stop=True,
                                         tile_position=(0, r0))
                    nc.vector.tensor_copy(ctxT[:, hp, :], ctx_ps[:])

                x1p = workA.tile([128, H], BF16, tag="x1p")
                proj_res(pp, ctxT, wsb["wo"], wsb["bo"], x0, x1p, 6)

                if not is_node:
                    nc.gpsimd.dma_start(utt_cls[2 * t:2 * t + 1, :],
                                        x1p[0:1, :])
                    nc.gpsimd.dma_start(utt_cls[2 * t + 1:2 * t + 2, :],
                                        x1p[64:65, :])
                    return

                x1 = workB.tile([128, H], BF16, tag="x1")
                layernorm(x1p, x1, wsb["g1"], wsb["b1"])
                x1T = transpose_ck(x1, 6, "xT6", eng=nc.scalar)
                x2p = workA.tile([128, H], BF16, tag="x1p")
                ffn(x1T, wsb, x1, x2p)
                x2n = workB.tile([128, H], BF16, tag="x2n")
                layernorm(x2p, x2n)

                # attention pool (g2/b2 folded: poolw = g2*pool_w)
                poolw_b, g2n_b, b2n_b = pool_env
                sc = small.tile([128, 1], F32, tag="poolsc")
                scratch = workA.tile([128, H], BF16, tag="xg")
                nc.vector.tensor_mul(scratch[:], x2n[:], poolw_b[:])
                nc.vector.reduce_sum(sc[:], scratch[:], axis=AX.X)
                poolT = pp1.tile([128, 128], F32, tag="ctx_ps")
                sp = poolT[0:1, :]
                nc.tensor.transpose(sp, sc[:, 0:1], id32[:])
                sp3 = sp.rearrange("a (n l) -> a n l", n=8)
                mx8 = small.tile([1, 8, 1], F32, tag="pmx")
                nc.vector.reduce_max(mx8[:], sp3, axis=AX.X)
                es = small.tile([1, 128], F32, tag="pes")
                es3 = es[:].rearrange("a (n l) -> a n l", n=8)
                nc.vector.tensor_tensor(out=es3, in0=sp3,
                                        in1=mx8[:].to_broadcast([1, 8, NL]),
                                        op=ALU.subtract)
                nc.scalar.activation(es[:], es[:], AF.Exp)
                sm8 = small.tile([1, 8, 1], F32, tag="psm")
                nc.vector.reduce_sum(sm8[:], es3, axis=AX.X)
                nc.vector.reciprocal(sm8[:], sm8[:])
                aw = small.tile([1, 128], F32, tag="paw")
                nc.vector.tensor_tensor(
                    out=aw[:].rearrange("a (n l) -> a n l", n=8), in0=es3,
                    in1=sm8[:].to_broadcast([1, 8, NL]), op=ALU.mult)
                awT_t = pp1.tile([128, 128], F32, tag="ctx_ps")
                nc.tensor.transpose(awT_t[:, 0:1], aw[0:1, :], id32[0:1, 0:1])
                ablk = workB.tile([128, 8], BF16, tag="ablk")
                nc.vector.tensor_scalar_mul(ablk[:], bm8[:], awT_t[:, 0:1])
                stage8 = workB.tile([8, H], BF16, tag="stage8")
                for g0, gw in ((0, 512), (512, 256)):
                    x0ps = pp.tile([8, 512], F32, tag="proj")
                    nc.tensor.matmul(x0ps[:, :gw], ablk[:],
                                     x2n[:, g0:g0 + gw], start=True, stop=True)
                    nc.vector.tensor_mul(stage8[:, g0:g0 + gw],
                                         x0ps[:, :gw], g2n_b[0:8, g0:g0 + gw])
                nc.vector.tensor_add(stage8[:], stage8[:], b2n_b[0:8, :])
                nl = t % 16
                nc.gpsimd.dma_start(x0sb[8 * nl:8 * nl + 8, t // 16, :],
                                    stage8[:])

            # ---------- utterance encoder ----------
            if "u" in phases:
                wsb = load_enc_weights("u")
                for t in range(UT_TILES):
                    enc_tile(wsb, d["emb_u"], t, is_node=False)

                g2u_b = encw.tile([128, H], BF16, tag="g2x")
                nc.sync.dma_start(g2u_b[:], d["g2u"].to_broadcast([128, H]))
                b2u_b = encw.tile([128, H], BF16, tag="b2x")
                nc.sync.dma_start(b2u_b[:], d["b2u"].to_broadcast([128, H]))
                x1c = workB.tile([16, H], BF16, tag="x1")
                layernorm(utt_cls, x1c, wsb["g1"], wsb["b1"], p=16)
                x1cT = workA.tile([128, 6, 16], BF16, tag="xT6")
                nc.sync.dma_start(x1cT[:], x1c[:], transpose=True)
                x2pc = workA.tile([16, H], BF16, tag="x1p")
                ffn(x1cT, wsb, x1c, x2pc, p=16)
                layernorm(x2pc, uttf, g2u_b, b2u_b, p=16)

            # ---------- node encoder ----------
            if "n" in phases:
                wsb = load_enc_weights("n")
                poolw_b = encw.tile([128, H], BF16, tag="poolw")
                nc.sync.dma_start(poolw_b[:],
                                  d["poolw"].to_broadcast([128, H]))
                g2n_b = encw.tile([128, H], BF16, tag="g2x")
                nc.sync.dma_start(g2n_b[:], d["g2n"].to_broadcast([128, H]))
                b2n_b = encw.tile([128, H], BF16, tag="b2x")
                nc.sync.dma_start(b2n_b[:], d["b2n"].to_broadcast([128, H]))

                env = (poolw_b, g2n_b, b2n_b)
                for t in range(ND_TILES):
                    enc_tile(wsb, d["emb_n"], t, is_node=True, pool_env=env)

        # ================= RGAT phase =================
        if "r" not in phases:
            _probe_out(nc, probe, dbg, uttf=uttf, x0sb=x0sb, x1sb=x1sb,
                       x2sb=x2sb, utt_cls=utt_cls)
            return
        with (
            tc.tile_pool(name="rgw", bufs=1) as rgw,
            tc.tile_pool(name="rstream", bufs=4) as rstream,
            tc.tile_pool(name="rwork", bufs=2) as rwork,
            tc.tile_pool(name="rpsum2", bufs=2, space="PSUM") as rp2,
            tc.tile_pool(name="rpsum1", bufs=1, space="PSUM") as rp1,
        ):
            adst_ne = rgw.tile([128, NODE_TILES, NEDGES], BF16)
            asrc_ne = rgw.tile([128, NODE_TILES, NEDGES], BF16)
            adst_en = rgw.tile([128, EDGE_TILES, NNODES], BF16)
            t1hot = rgw.tile([128, EDGE_TILES, R], BF16)
            nc.sync.dma_start(adst_ne[:],
                              d["AdstNE"].rearrange("(n p) e -> p n e", p=128))
            nc.sync.dma_start(asrc_ne[:],
                              d["AsrcNE"].rearrange("(n p) e -> p n e", p=128))
            nc.sync.dma_start(adst_en[:],
                              d["AdstEN"].rearrange("(n p) e -> p n e", p=128))
            nc.sync.dma_start(t1hot[:],
                              d["T1hot"].rearrange("(n p) e -> p n e", p=128))
            asrcr = rgw.tile([128, R, NODE_TILES, NEDGES], BF16)
            for r in range(R):
                nc.sync.dma_start(
                    asrcr[:, r, :, :],
                    d["AsrcR"][ds(r * NNODES, NNODES), :].rearrange(
                        "(n p) e -> p n e", p=128))

            def conv(x_in, nck, hin, hout, Wd, Ud, Vd, x_out):
                xT = rwork.tile([128, NODE_TILES, nck, 128], BF16, tag="rxT")
                for ntb in range(NODE_TILES):
                    nc.sync.dma_start(xT[:, ntb, :, :], x_in[:, ntb, :],
                                      transpose=True)
                uv = rstream.tile([128, nck, 2 * R], BF16, tag="ruv")
                nc.sync.dma_start(uv[:, :, 0:R],
                                  Ud.rearrange("(c p) r -> p c r", p=128))
                nc.sync.dma_start(uv[:, :, R:2 * R],
                                  Vd.rearrange("(c p) r -> p c r", p=128))
                psb = rwork.tile([128, NODE_TILES, 2 * R], BF16, tag="rpq")
                for ntb in range(NODE_TILES):
                    pq = rp1.tile([128, 2 * R], F32, tag="rpq_ps")
                    for c in range(nck):
                        nc.tensor.matmul(pq[:], xT[:, ntb, c, :], uv[:, c, :],
                                         start=(c == 0), stop=(c == nck - 1))
                    nc.vector.tensor_copy(psb[:, ntb, :], pq[:])
                ysb = rgw.tile([128, NODE_TILES, R, hout], BF16, tag="rY")
                for r in range(R):
                    wrt = rstream.tile([128, nck, hout], BF16, tag="rwr")
                    nc.sync.dma_start(
                        wrt[:],
                        Wd[ds(r * hin, hin), :].rearrange(
                            "(c p) f -> p c f", p=128))
                    for ntb in range(NODE_TILES):
                        yp = rp2.tile([128, hout], F32, tag="ry_ps")
                        for c in range(nck):
                            nc.tensor.matmul(yp[:], xT[:, ntb, c, :],
                                             wrt[:, c, :], start=(c == 0),
                                             stop=(c == nck - 1))
                        nc.vector.tensor_copy(ysb[:, ntb, r, :], yp[:])
                expE = rwork.tile([128, EDGE_TILES], F32, tag="rexpE")
                for etb in range(EDGE_TILES):
                    eps = rp1.tile([128, R], F32, tag="re_ps")
                    for ntb in range(NODE_TILES):
                        nc.tensor.matmul(eps[:],
                                         adst_ne[:, ntb, ts(etb, 128)],
                                         psb[:, ntb, 0:R],
                                         start=(ntb == 0), stop=False)
                    for ntb in range(NODE_TILES):
                        nc.tensor.matmul(eps[:],
                                         asrc_ne[:, ntb, ts(etb, 128)],
                                         psb[:, ntb, R:2 * R],
                                         start=False, stop=(ntb == 2))
                    scr = rwork.tile([128, R], BF16, tag="rescr")
                    eraw = small.tile([128, 1], F32, tag="reraw")
                    nc.vector.tensor_mul(scr[:], eps[:], t1hot[:, etb, :])
                    nc.vector.reduce_sum(eraw[:], scr[:], axis=AX.X)
                    nc.scalar.activation(eraw[:], eraw[:], AF.Lrelu, alpha=0.2)
                    nc.scalar.activation(expE[:, etb:etb + 1], eraw[:], AF.Exp)
                wmsg = rwork.tile([128, EDGE_TILES, hout + 1], BF16,
                                  tag="rwmsg")
                for etb in range(EDGE_TILES):
                    mps = rp2.tile([128, hout], F32, tag="rm_ps")
                    first = True
                    for r in range(R):
                        for ntb in range(NODE_TILES):
                            nc.tensor.matmul(mps[:],
                                             asrcr[:, r, ntb, ts(etb, 128)],
                                             ysb[:, ntb, r, :], start=first,
                                             stop=(r == R - 1 and ntb == 2))
                            first = False
                    nc.vector.tensor_scalar_mul(wmsg[:, etb, 0:hout], mps[:],
                                                expE[:, etb:etb + 1])
                    nc.vector.tensor_copy(wmsg[:, etb, hout:hout + 1],
                                          expE[:, etb:etb + 1])
                for ntb in range(NODE_TILES):
                    aps = rp1.tile([128, hout + 1], F32, tag="ragg_ps")
                    for etb in range(EDGE_TILES):
                        nc.tensor.matmul(aps[:],
                                         adst_en[:, etb, ts(ntb, 128)],
                                         wmsg[:, etb, :],
                                         start=(etb == 0),
                                         stop=(etb == EDGE_TILES - 1))
                    rden = small.tile([128, 1], F32, tag="rden")
                    nc.vector.tensor_scalar_max(rden[:],
                                                aps[:, hout:hout + 1], 1e-30)
                    nc.vector.reciprocal(rden[:], rden[:])
                    nc.vector.tensor_scalar_mul(x_out[:, ntb, :],
                                                aps[:, 0:hout], rden[:])

            conv(x0sb, 6, H, H1, d["W1"], d["U1"], d["V1"], x1sb)
            conv(x1sb, 2, H1, H2, d["W2"], d["U2"], d["V2"], x2sb)

        # ================= classifier phase =================
        if "c" not in phases:
            _probe_out(nc, probe, dbg, uttf=uttf, x0sb=x0sb, x1sb=x1sb,
                       x2sb=x2sb, utt_cls=utt_cls)
            return
        with (
            tc.tile_pool(name="cw", bufs=1) as cw,
            tc.tile_pool(name="cpsum", bufs=2, space="PSUM") as cp,
        ):
            meanT = cw.tile([128, NODE_TILES, TPC], BF16)
            nc.sync.dma_start(meanT[:],
                              d["meanT"].rearrange("(n p) t -> p n t", p=128))
            ip = cp.tile([16, H2], F32, tag="inner_ps")
            for ntb in range(NODE_TILES):
                nc.tensor.matmul(ip[:], meanT[:, ntb, :], x2sb[:, ntb, :],
                                 start=(ntb == 0), stop=(ntb == NODE_TILES - 1))
            inner = cw.tile([16, H2], BF16)
            nc.vector.tensor_copy(inner[:], ip[:])

            featT = cw.tile([128, 8, 16], BF16)
            nc.sync.dma_start(featT[:, 0:6, :], uttf[:], transpose=True)
            nc.sync.dma_start(featT[:, 6:8, :], inner[:], transpose=True)

            w1c = cw.tile([128, 8, CLF], BF16)
            nc.sync.dma_start(w1c[:],
                              d["w1c"].rearrange("(c p) f -> p c f", p=128))
            w2c = cw.tile([128, 6, CLF], BF16)
            nc.sync.dma_start(w2c[:],
                              d["w2c"].rearrange("(c p) f -> p c f", p=128))
            w3c = cw.tile([128, 6, LBL], BF16)
            nc.sync.dma_start(w3c[:],
                              d["w3c"].rearrange("(c p) f -> p c f", p=128))
            bcs = {}
            for bn, dim in [("b1c", CLF), ("b2c", CLF), ("b3c", LBL)]:
                t = cw.tile([1, dim], BF16, tag=bn)
                nc.sync.dma_start(t[:], d[bn][:])
                bcs[bn] = t
            lblh = cw.tile([16, LBL], F32)
            nc.sync.dma_start(lblh[:], d["lblhot"][:])

            z1 = cw.tile([16, CLF], BF16)
            proj(cp, featT, w1c, bcs["b1c"], z1, 8, CLF, act=AF.Relu, p=16)
            z1T = cw.tile([128, 6, 16], BF16)
            nc.sync.dma_start(z1T[:], z1[:], transpose=True)
            z2 = cw.tile([16, CLF], BF16)
            proj(cp, z1T, w2c, bcs["b2c"], z2, 6, CLF, act=AF.Relu, p=16)
            z2T = cw.tile([128, 6, 16], BF16)
            nc.sync.dma_start(z2T[:], z2[:], transpose=True)
            lp = cp.tile([16, LBL], F32, tag="logit_ps")
            nc.tensor.matmul(lp[:], ones1[:, :16], bcs["b3c"][:],
                             start=True, stop=False)
            for c in range(6):
                nc.tensor.matmul(lp[:], z2T[:, c, :], w3c[:, c, :],
                                 start=False, stop=(c == 5))

            osb = cw.tile([16, 8], F32)
            nc.vector.tensor_copy(osb[:, 0:LBL], lp[:])
            mxl = small.tile([16, 1], F32, tag="cmx")
            nc.vector.reduce_max(mxl[:], lp[:], axis=AX.X)
            negml = small.tile([16, 1], F32, tag="cnegm")
            nc.vector.tensor_scalar_mul(negml[:], mxl[:], -1.0)
            el = cw.tile([16, LBL], F32)
            denl = small.tile([16, 1], F32, tag="cden")
            nc.scalar.activation(el[:], lp[:], AF.Exp, bias=negml[:],
                                 accum_out=denl[:])
            lsd = small.tile([16, 1], F32, tag="clsd")
            nc.scalar.activation(lsd[:], denl[:], AF.Ln)
            nc.vector.tensor_add(lsd[:], lsd[:], mxl[:])
            pick = small.tile([16, 1], F32, tag="cpick")
            pscr = cw.tile([16, LBL], F32)
            nc.vector.tensor_mul(pscr[:], lp[:], lblh[:])
            nc.vector.reduce_sum(pick[:], pscr[:], axis=AX.X)
            nc.vector.tensor_sub(osb[:, 7:8], lsd[:], pick[:])
            nc.sync.dma_start(out[:], osb[:])
        _probe_out(nc, probe, dbg, uttf=uttf, x0sb=x0sb, x1sb=x1sb,
                   x2sb=x2sb, utt_cls=utt_cls)


def _probe_out(nc, probe, dbg, **tiles):
    if probe is None:
        return
    t = tiles[probe]
    if probe in ("uttf", "utt_cls"):
        nc.gpsimd.dma_start(dbg[0:16, 0:H], t[:])
    elif probe == "x0sb":
        nc.gpsimd.dma_start(dbg[:], t[:].rearrange("p a b -> p (a b)"))
    else:  # x1sb / x2sb [128, 3, 256]
        nc.gpsimd.dma_start(dbg[:, 0:3 * 256],
                            t[:].rearrange("p a b -> p (a b)"))


# ======================================================================
# Host-side preparation
# ======================================================================

def _prep_shared(params):
    p = {k: {kk: np.asarray(vv, np.float32) for kk, vv in v.items()}
         if isinstance(v, dict) else np.asarray(v, np.float32)
         for k, v in params.items()}
    sh = {}
    for enc, key in (("u", "utt"), ("n", "node")):
        e = p[key]
        sh["emb_" + enc] = _bf(e["emb"])
        seqlen = S if enc == "u" else NL
        sh["pos_" + enc] = _bf(np.tile(e["pos"][:seqlen], (128 // seqlen, 1)))
        for w in ("wq", "wk", "wv", "wo"):
            sh[w + enc] = _bf(e[w])
        sh["w1" + enc] = _bf(e["w1"])
        sh["w2" + enc] = _bf(e["w2"])
        sh["bqf" + enc] = np.ascontiguousarray(
            e["bq"].reshape(6, 128).T.astype(np.float32))
        sh["bkf" + enc] = np.ascontiguousarray(
            e["bk"].reshape(6, 128).T.astype(np.float32))
        sh["bf1f" + enc] = np.ascontiguousarray(
            e["bf1"].reshape(24, 128).T.astype(np.float32))
        for bn in ("bv", "bo", "bf2"):
            sh[bn + enc] = _bf(e[bn][None])
        for gn, src in (("g0", "ln0_g"), ("b0", "ln0_b"), ("g1", "ln1_g"),
                        ("b1", "ln1_b")):
            sh[gn + enc] = _bf(e[src][None])
    sh["g2u"] = _bf(p["utt"]["ln2_g"][None])
    sh["b2u"] = _bf(p["utt"]["ln2_b"][None])
    sh["g2n"] = _bf(p["node"]["ln2_g"][None])
    sh["b2n"] = _bf(p["node"]["ln2_b"][None])
    sh["poolw"] = _bf((p["node"]["ln2_g"] * p["pool_w"][:, 0])[None])

    def blockmask(bs):
        m = np.full((128, 128), -1e9, np.float32)
        for s0 in range(0, 128, bs):
            m[s0:s0 + bs, s0:s0 + bs] = 0.0
        return m
    sh["mask_u"] = blockmask(S)
    sh["mask_n"] = blockmask(NL)
    bm8 = np.zeros((128, 8), np.float32)
    for t in range(128):
        bm8[t, t // NL] = 1.0
    sh["bm8"] = _bf(bm8)

    for i, (cv, hin) in enumerate((("conv1", H), ("conv2", H1))):
        c = p[cv]
        W = np.einsum("rb,bio->rio", c["comp"], c["basis"]).astype(np.float32)
        sh[f"W{i + 1}"] = _bf(W.reshape(R * hin, -1))
        sh[f"U{i + 1}"] = _bf(np.einsum("rio,o->ir", W, c["aq"]))
        sh[f"V{i + 1}"] = _bf(np.einsum("rio,o->ir", W, c["ak"]))

    sh["w1c"] = _bf(p["clf_w1"])
    sh["w2c"] = _bf(p["clf_w2"])
    sh["w3c"] = _bf(p["clf_w3"])
    sh["b1c"] = _bf(p["clf_b1"][None])
    sh["b2c"] = _bf(p["clf_b2"][None])
    sh["b3c"] = _bf(p["clf_b3"][None])

    meanT = np.zeros((NNODES, TPC), np.float32)
    for n in range(NNODES):
        meanT[n, n // NPT] = 1.0 / NPT
    sh["meanT"] = _bf(meanT)
    return sh


def _prep_core(c, input_ids, node_input_ids, edge_index, edge_types, labels):
    m = {}
    m["utt_ids"] = np.asarray(input_ids[c], np.int32).reshape(UT_TOK, 1)
    m["node_ids"] = np.asarray(node_input_ids[c], np.int32).reshape(ND_TOK, 1)

    ei = np.asarray(edge_index[c], np.int64)
    et = np.asarray(edge_types[c], np.int64)
    src = (ei[..., 0] + np.arange(TPC)[:, None] * NPT).reshape(-1)
    dst = (ei[..., 1] + np.arange(TPC)[:, None] * NPT).reshape(-1)
    etf = et.reshape(-1)
    eids = np.arange(NEDGES)
    adst = np.zeros((NNODES, NEDGES), np.float32)
    adst[dst, eids] = 1.0
    asrc = np.zeros((NNODES, NEDGES), np.float32)
    asrc[src, eids] = 1.0
    m["AdstNE"] = _bf(adst)
    m["AsrcNE"] = _bf(asrc)
    m["AdstEN"] = _bf(adst.T)
    asrcr = np.zeros((R, NNODES, NEDGES), np.float32)
    asrcr[etf, src, eids] = 1.0
    m["AsrcR"] = _bf(asrcr.reshape(R * NNODES, NEDGES))
    t1 = np.zeros((NEDGES, R), np.float32)
    t1[eids, etf] = 1.0
    m["T1hot"] = _bf(t1)

    lbl = np.asarray(labels[c], np.int64)
    lh = np.zeros((TPC, LBL), np.float32)
    lh[np.arange(TPC), lbl] = 1.0
    m["lblhot"] = lh
    return m


_NC_CACHE = None


def _get_nc():
    global _NC_CACHE
    if _NC_CACHE is None:
        _NC_CACHE = _build_graph()
    return _NC_CACHE


def kernel(input_ids, attention_mask, node_input_ids, node_attention_mask,
           edge_index, edge_types, labels, params):
    nc = _get_nc()
    sh = _prep_shared(params)
    input_ids = np.asarray(input_ids)
    node_input_ids = np.asarray(node_input_ids)
    edge_index = np.asarray(edge_index)
    edge_types = np.asarray(edge_types)
    labels = np.asarray(labels)

    in_maps = []
    for c in range(NCORES):
        m = dict(sh)
        m.update(_prep_core(c, input_ids, node_input_ids, edge_index,
                            edge_types, labels))
        in_maps.append(m)

    trace = bool(int(os.environ.get("KERNEL_TRACE", "0")))
    res = run_bass_kernel_spmd(nc, in_maps, core_ids=list(range(NCORES)),
                               trace=trace)
    if trace:
        kernel.last_exec_time_ns = res.exec_time_ns

    outs = [res.results[c]["out"] for c in range(NCORES)]
    logits = np.concatenate([o[:, :LBL] for o in outs], axis=0)
    loss = np.float32(np.mean(np.concatenate([o[:, 7] for o in outs])))
    return logits.astype(np.float32), loss
